# revision 9
# baseline (speedup 1.0000x reference)
"""ConvNeXt block kernel for Trainium2 (8 NeuronCores, batch-parallel).

Computes, for x:[B,C,L]:
  p   = depthwise_conv1d(x, dw_w, k=7, pad=3) + dw_b          (per-channel)
  n   = LayerNorm(p.transpose(0,2,1), normalized over [L,C])  (per-batch scalar stats)
  h   = gelu(n @ w1.T + b1)                                   (exact erf gelu)
  y   = h @ w2.T + b2 + x
Sharding: data-parallel over batch, B=16 -> 2 batches per core, no collectives.

Schedule (bf16 matmul roofline is ~109us/core; baseline measured 175us):
  - b0/lc0's depthwise conv runs ON THE PE as 7 accumulating diagonal
    matmuls per c-tile (stationary = diag(dw_w[:,k]) blocks, moving = the
    shifted x window).  That kills the otherwise-exposed DVE conv latency
    at kernel start: the PE convolves while DVE convs b0/lc1 + b1 in the
    shadow of later GEMM phases (plain scalar_tensor_tensor chains --
    they have no DVE 2x mode, but nothing is latency-bound on them).
  - dw_b rides the ACT eviction of the conv PSUM (per-partition bias),
    which also emits the row-sum accumulator used for the LN mean.
  - LN stats never touch the PE (gpsimd partition_all_reduce broadcasts
    the scalar sums; the rsqrt Newton chain runs on all 128 partitions).
    LN normalization stays folded into the GEMM1 epilogue:
    h = rs*(p @ w1.T) + (bcoef*rowsum(w1) + b1).
  - GELU for b0/lc0 is stats-gated, so that GEMM1 output is evicted
    PSUM->SBUF (bf16) on ACT; b0/lc1 and all of b1 gelu straight out of
    PSUM.  GEMM2 for b1 is software-pipelined one h-tile behind GEMM1.
  - DMA descriptor-gen costs ~0.64us per transfer on the issuing engine,
    so bulk loads are split between the sync and gpsimd queues and the
    y stores alternate between them.
  - ~50 dummy N=128 matmuls on a zero tile warm the PE HAM clock gate
    during the first x/weight DMAs.
"""

import sys

if "/opt/trn_rl_repo" not in sys.path:
    sys.path.insert(0, "/opt/trn_rl_repo")

import numpy as np

P = 128
B, C, L, H = 16, 512, 1024, 2048
KW = 7
PAD = 3
CT = C // P          # 4 c-tiles
HT = H // P          # 16 h-tiles
LCW = 512            # l-chunk width (one PSUM bank of fp32)
NLC = L // LCW       # 2 l-chunks
N_CORES = 8
BPC = B // N_CORES   # 2 batches per core
N_ELEMS = float(C * L)
LN_EPS = 1e-5
XW = L + 8           # padded x row: [0]*4 + x + [0]*4
EPAD = 4             # x offset inside the even-parity tile
OPAD = 3             # x offset inside the odd-parity tile
DMASPLIT = 520       # column where x DMAs are split across queues
N_WARM_MM = 50       # dummy matmuls to warm the PE HAM clock gate

_prog_cache = {}


def _build_program(sim_act=False):
    """sim_act=True swaps Gelu -> Tanh (CoreSim has no Gelu table); dev only."""
    from contextlib import ExitStack

    from concourse import bacc, bass_isa, mybir, tile
    from concourse.alu_op_type import AluOpType

    f32 = mybir.dt.float32
    bf16 = mybir.dt.bfloat16
    AF = mybir.ActivationFunctionType
    AX = mybir.AxisListType
    RED = bass_isa.ReduceOp

    nc = bacc.Bacc("TRN2", target_bir_lowering=False, debug=False,
                   num_devices=N_CORES)

    xb_d = nc.dram_tensor("xb", [BPC, C, XW], bf16, kind="ExternalInput").ap()
    dgt_d = nc.dram_tensor("dgt", [P, CT * KW * P], bf16,
                           kind="ExternalInput").ap()
    dww_d = nc.dram_tensor("dww", [P, CT * KW], f32, kind="ExternalInput").ap()
    dwb_d = nc.dram_tensor("dwb", [P, CT], f32, kind="ExternalInput").ap()
    w1t_d = nc.dram_tensor("w1t", [C, H], bf16, kind="ExternalInput").ap()
    b1s_d = nc.dram_tensor("b1s", [P, HT], f32, kind="ExternalInput").ap()
    s1s_d = nc.dram_tensor("s1s", [P, HT], f32, kind="ExternalInput").ap()
    w2t_d = nc.dram_tensor("w2t", [H, C], bf16, kind="ExternalInput").ap()
    b2s_d = nc.dram_tensor("b2s", [P, CT], f32, kind="ExternalInput").ap()
    y_d = nc.dram_tensor("y", [BPC, C, L], f32, kind="ExternalOutput").ap()

    with tile.TileContext(nc) as tc, ExitStack() as ctx:
        const = ctx.enter_context(tc.tile_pool(name="const", bufs=1))
        wpool = ctx.enter_context(tc.tile_pool(name="wts", bufs=1))
        xpool = ctx.enter_context(tc.tile_pool(name="xp", bufs=1))
        ppool = ctx.enter_context(tc.tile_pool(name="pp", bufs=1))
        apool = ctx.enter_context(tc.tile_pool(name="acc", bufs=2))
        stp = ctx.enter_context(tc.tile_pool(name="stats", bufs=1))
        scr = ctx.enter_context(tc.tile_pool(name="scratch", bufs=2))
        gpool = ctx.enter_context(tc.tile_pool(name="g", bufs=16))
        ypool = ctx.enter_context(tc.tile_pool(name="yo", bufs=4))
        hpool = ctx.enter_context(tc.tile_pool(name="hpre", bufs=16))
        ps_h = ctx.enter_context(tc.tile_pool(name="psh", bufs=3, space="PSUM"))
        ps_y = ctx.enter_context(tc.tile_pool(name="psy", bufs=4, space="PSUM"))
        ps_w = ctx.enter_context(tc.tile_pool(name="psw", bufs=1, space="PSUM"))

        # ---- HAM warmup (PE busy during the first DMAs) ----
        wz = const.tile([P, P], bf16, tag="wz")
        nc.vector.memset(wz[:], 0.0)
        warm = ps_w.tile([P, P], f32, tag="warm", name="warm")
        for i in range(N_WARM_MM):
            nc.tensor.matmul(warm[:], wz[:], wz[:], start=True, stop=True)

        # ---- DMAs.  Descriptor-gen is ~0.64us per transfer on the issuing
        # engine, so split: sync loads what the PE needs first (diag conv
        # weights + b0's x, then w1); gpsimd loads b1's x and w2.
        xe = {}
        dgt = const.tile([P, CT * KW * P], bf16, tag="dgt")
        for ct in range(CT):
            nc.sync.dma_start(
                out=dgt[:, ct * KW * P:(ct + 1) * KW * P],
                in_=dgt_d[:, ct * KW * P:(ct + 1) * KW * P])
            t = xpool.tile([P, XW], bf16, tag=f"xe_0_{ct}", name=f"xe_0_{ct}")
            nc.sync.dma_start(out=t[:], in_=xb_d[0, ct * P:(ct + 1) * P, :])
            xe[0, ct] = t
        dww = const.tile([P, CT * KW], f32, tag="dww")
        nc.sync.dma_start(out=dww[:], in_=dww_d[:])
        dwb = const.tile([P, CT], f32, tag="dwb")
        nc.sync.dma_start(out=dwb[:], in_=dwb_d[:])
        w1 = []
        for ct in range(CT):
            w = wpool.tile([P, H], bf16, tag=f"w1_{ct}")
            nc.sync.dma_start(out=w[:], in_=w1t_d[ct * P:(ct + 1) * P, :])
            w1.append(w)
        b1s = const.tile([P, HT], f32, tag="b1s")
        nc.sync.dma_start(out=b1s[:], in_=b1s_d[:])
        s1s = const.tile([P, HT], f32, tag="s1s")
        nc.sync.dma_start(out=s1s[:], in_=s1s_d[:])
        b2s = const.tile([P, CT], f32, tag="b2s")
        nc.sync.dma_start(out=b2s[:], in_=b2s_d[:])

        for ct in range(CT):
            t = xpool.tile([P, XW], bf16, tag=f"xe_1_{ct}", name=f"xe_1_{ct}")
            nc.gpsimd.dma_start(out=t[:], in_=xb_d[1, ct * P:(ct + 1) * P, :])
            xe[1, ct] = t
        w2 = []
        for ht in range(HT):
            w = wpool.tile([P, C], bf16, tag=f"w2_{ht}")
            nc.gpsimd.dma_start(out=w[:], in_=w2t_d[ht * P:(ht + 1) * P, :])
            w2.append(w)

        # ---- conv + stats machinery ----
        # stats[b] cols: [0:nm) per-(lc,ct) row sums, [nm:2nm) square sums
        NM = {0: 2 * CT, 1: CT}
        stats = {}
        pb = {}
        for b in range(BPC):
            stats[b] = stp.tile([P, 2 * NM[b]], f32, tag=f"st_{b}",
                                name=f"st_{b}")
            pb[b] = [ppool.tile([P, L], bf16, tag=f"p_{b}_{ct}",
                                name=f"p_{b}_{ct}") for ct in range(CT)]

        def conv_chunk_dve(b, o, n, mcol0):
            """7-tap STT chain per c-tile; final tap carries the row-sum
            accumulator for the LN mean."""
            for ct in range(CT):
                acc = apool.tile([P, n], bf16, tag=f"acc{n}",
                                 name=f"acc_{b}_{o}_{ct}")
                nc.vector.tensor_scalar(
                    acc[:], xe[b, ct][:, EPAD + o:EPAD + o + n],
                    dww[:, ct * KW + PAD:ct * KW + PAD + 1],
                    dwb[:, ct:ct + 1], AluOpType.mult, AluOpType.add)
                for j in (-3, -2, -1, 1, 2):
                    nc.vector.scalar_tensor_tensor(
                        acc[:], xe[b, ct][:, EPAD + o + j:EPAD + o + j + n],
                        dww[:, ct * KW + PAD + j:ct * KW + PAD + j + 1],
                        acc[:], AluOpType.mult, AluOpType.add)
                mcol = mcol0 + ct
                nc.vector.scalar_tensor_tensor(
                    pb[b][ct][:, o:o + n],
                    xe[b, ct][:, EPAD + o + 3:EPAD + o + 3 + n],
                    dww[:, ct * KW + PAD + 3:ct * KW + PAD + 3 + 1],
                    acc[:], AluOpType.mult, AluOpType.add,
                    accum_out=stats[b][:, mcol:mcol + 1])

        def emit_squares(b, o, n, scol0):
            for ct in range(CT):
                sq = scr.tile([P, n], f32, tag=f"sq{n}",
                              name=f"sq_{b}_{o}_{ct}")
                scol = scol0 + ct
                nc.scalar.activation(sq[:], pb[b][ct][:, o:o + n], AF.Square,
                                     accum_out=stats[b][:, scol:scol + 1])

        ab = {}
        bias16 = {}

        def stats_chain(b):
            nm = NM[b]
            hp_ctx = tc.high_priority()
            hp_ctx.__enter__()
            sq2 = stp.tile([P, 2], f32, tag=f"sq2_{b}", name=f"sq2_{b}")
            nc.vector.tensor_reduce(sq2[:, 0:1], stats[b][:, 0:nm], AX.X,
                                    AluOpType.add)
            nc.vector.tensor_reduce(sq2[:, 1:2], stats[b][:, nm:2 * nm], AX.X,
                                    AluOpType.add)
            alr = stp.tile([P, 2], f32, tag=f"alr_{b}", name=f"alr_{b}")
            nc.gpsimd.partition_all_reduce(alr[:], sq2[:], 128, RED.add)

            e = stp.tile([P, 4], f32, tag=f"e_{b}", name=f"e_{b}")
            nc.vector.tensor_scalar(e[:, 0:2], alr[:], 1.0 / N_ELEMS,
                                    None, AluOpType.mult)
            nc.vector.scalar_tensor_tensor(e[:, 2:3], e[:, 0:1], -1.0,
                                           e[:, 0:1], AluOpType.mult,
                                           AluOpType.mult)
            nc.vector.scalar_tensor_tensor(e[:, 3:4], e[:, 1:2], LN_EPS,
                                           e[:, 2:3], AluOpType.add,
                                           AluOpType.add)
            # rs = rsqrt(var+eps) on DVE (magic seed + 2 Newton steps)
            nt = stp.tile([P, 8], f32, tag=f"nt_{b}", name=f"nt_{b}")
            i32 = mybir.dt.int32
            v = e[:, 3:4]
            nc.vector.tensor_scalar(nt[:, 0:1].bitcast(i32), v.bitcast(i32),
                                    1, None, AluOpType.arith_shift_right)
            nc.vector.tensor_scalar(nt[:, 1:2].bitcast(i32),
                                    nt[:, 0:1].bitcast(i32), -1, 0x5F3759DF,
                                    AluOpType.mult, AluOpType.add)
            nc.vector.tensor_scalar(nt[:, 2:3], v, -0.5, None, AluOpType.mult)
            r, hv = nt[:, 1:2], nt[:, 2:3]
            abt = stp.tile([P, 2], f32, tag=f"ab_{b}", name=f"ab_{b}")
            for it in range(2):
                nc.vector.tensor_tensor(nt[:, 3:4], r, r, AluOpType.mult)
                nc.vector.tensor_tensor(nt[:, 4:5], nt[:, 3:4], hv,
                                        AluOpType.mult)
                nc.vector.tensor_scalar(nt[:, 5:6], nt[:, 4:5], 1.5, None,
                                        AluOpType.add)
                dst = nt[:, 6:7] if it < 1 else abt[:, 0:1]
                nc.vector.tensor_tensor(dst, r, nt[:, 5:6], AluOpType.mult)
                r = nt[:, 6:7]
            nc.vector.scalar_tensor_tensor(abt[:, 1:2], e[:, 0:1], -1.0,
                                           abt[:, 0:1], AluOpType.mult,
                                           AluOpType.mult)    # -mu*rs
            bt = stp.tile([P, HT], f32, tag=f"b16_{b}", name=f"b16_{b}")
            nc.vector.scalar_tensor_tensor(bt[:], s1s[:], abt[:, 1:2],
                                           b1s[:], AluOpType.mult,
                                           AluOpType.add)
            hp_ctx.__exit__(None, None, None)
            ab[b] = abt
            bias16[b] = bt

        act_fn = AF.Tanh if sim_act else AF.Gelu

        def gemm1_group(b, ht, o, n, out_ap):
            for ct in range(CT):
                nc.tensor.matmul(out_ap,
                                 w1[ct][:, ht * P:(ht + 1) * P],
                                 pb[b][ct][:, o:o + n],
                                 start=(ct == 0), stop=(ct == CT - 1))

        def gemm2_group(b, ht, g_ap, pys, start, stop):
            for ct in range(CT):
                nc.tensor.matmul(pys[ct][:],
                                 w2[ht][:, ct * P:(ct + 1) * P],
                                 g_ap, start=start, stop=stop)

        def epilogue(b, lc, pys):
            o = lc * LCW
            for ct in range(CT):
                yt = ypool.tile([P, LCW], f32, tag="yt",
                                name=f"yt_{b}_{lc}_{ct}")
                nc.vector.scalar_tensor_tensor(
                    yt[:], pys[ct][:], b2s[:, ct:ct + 1],
                    xe[b, ct][:, EPAD + o:EPAD + o + LCW],
                    AluOpType.add, AluOpType.add)
                r0 = ct * P
                eng = nc.sync if ct < 2 else nc.gpsimd
                eng.dma_start(out=y_d[b, r0:r0 + P, o:o + LCW], in_=yt[:])

        # ================= batch 0 =================
        # DVE convs first in emission so they fill the DVE stream early;
        # all of their output is consumed much later.
        conv_chunk_dve(0, LCW, LCW, CT)      # b0/lc1
        conv_chunk_dve(1, 0, L, 0)           # all of b1

        # b0/lc0 conv on the PE: 7 accumulating diagonal matmuls per c-tile
        cps = []
        for ct in range(CT):
            cp = ps_y.tile([P, LCW], f32, tag="py", name=f"cp_{ct}")
            for k in range(KW):
                j = k - PAD
                nc.tensor.matmul(cp[:],
                                 dgt[:, (ct * KW + k) * P:(ct * KW + k + 1) * P],
                                 xe[0, ct][:, EPAD + j:EPAD + j + LCW],
                                 start=(k == 0), stop=(k == KW - 1))
            cps.append(cp)
        # evict conv PSUM -> pb (bf16), adding dw_b and emitting the row sum
        for ct in range(CT):
            nc.scalar.activation(pb[0][ct][:, 0:LCW], cps[ct][:],
                                 AF.Identity, bias=dwb[:, ct:ct + 1],
                                 accum_out=stats[0][:, ct:ct + 1])
        emit_squares(0, 0, LCW, 2 * CT)      # lc0 squares

        # GEMM1 lc0 (stats-gated GELU -> evict to SBUF)
        hp = [hpool.tile([P, LCW], bf16, tag="hp", name=f"hp_{ht}")
              for ht in range(HT)]
        for ht in range(HT):
            ph = ps_h.tile([P, LCW], f32, tag="ph", name=f"ph_w_{ht}")
            gemm1_group(0, ht, 0, LCW, ph[:])
            nc.scalar.copy(hp[ht][:], ph[:])
            if ht == 7:
                # slot the lc1 squares into the ACT stream right when the
                # DVE conv for lc1 lands, so stats close as early as possible
                emit_squares(0, LCW, LCW, 2 * CT + CT)
                stats_chain(0)

        # GEMM1 lc1 with live GELU from PSUM, then GEMM2 lc1
        g_lc1 = []
        for ht in range(HT):
            ph = ps_h.tile([P, LCW], f32, tag="ph", name=f"ph_0_1_{ht}")
            gemm1_group(0, ht, LCW, LCW, ph[:])
            g = gpool.tile([P, LCW], bf16, tag="g1", name=f"g_0_1_{ht}")
            nc.scalar.activation(g[:], ph[:], act_fn,
                                 bias=bias16[0][:, ht:ht + 1],
                                 scale=ab[0][:, 0:1])
            g_lc1.append(g)
        pys = [ps_y.tile([P, LCW], f32, tag="py", name=f"py_0_1_{i}")
               for i in range(CT)]
        for ht in range(HT):
            gemm2_group(0, ht, g_lc1[ht][:], pys, ht == 0, ht == HT - 1)
        epilogue(0, 1, pys)

        # GELU lc0 from the evicted tiles, then GEMM2 lc0
        g_lc0 = []
        for ht in range(HT):
            g = gpool.tile([P, LCW], bf16, tag="g0", name=f"g_0_0_{ht}")
            nc.scalar.activation(g[:], hp[ht][:], act_fn,
                                 bias=bias16[0][:, ht:ht + 1],
                                 scale=ab[0][:, 0:1])
            g_lc0.append(g)
        pys = [ps_y.tile([P, LCW], f32, tag="py", name=f"py_0_0_{i}")
               for i in range(CT)]
        for ht in range(HT):
            gemm2_group(0, ht, g_lc0[ht][:], pys, ht == 0, ht == HT - 1)
        epilogue(0, 0, pys)

        # ================= batch 1 =================
        emit_squares(1, 0, L, CT)
        stats_chain(1)
        for lc in range(NLC):
            pys = [ps_y.tile([P, LCW], f32, tag="py", name=f"py_1_{lc}_{i}")
                   for i in range(CT)]
            prev = None
            for ht in range(HT):
                ph = ps_h.tile([P, LCW], f32, tag="ph",
                               name=f"ph_1_{lc}_{ht}")
                gemm1_group(1, ht, lc * LCW, LCW, ph[:])
                g = gpool.tile([P, LCW], bf16, tag=f"g{lc}",
                               name=f"g_1_{lc}_{ht}")
                nc.scalar.activation(g[:], ph[:], act_fn,
                                     bias=bias16[1][:, ht:ht + 1],
                                     scale=ab[1][:, 0:1])
                if prev is not None:
                    gemm2_group(1, prev[0], prev[1][:], pys,
                                prev[0] == 0, False)
                prev = (ht, g)
            gemm2_group(1, prev[0], prev[1][:], pys, False, True)
            epilogue(1, lc, pys)

    nc.compile()
    return nc


def _get_program():
    if "nc" not in _prog_cache:
        _prog_cache["nc"] = _build_program()
    return _prog_cache["nc"]


def _pack_inputs(x, dw_w, dw_b, w1, b1, w2, b2):
    """Host-side packing into the per-core DRAM tensor layouts."""
    import ml_dtypes

    bf = ml_dtypes.bfloat16
    x = np.ascontiguousarray(x, dtype=np.float32)
    xb = np.zeros((B, C, XW), dtype=bf)
    xb[:, :, EPAD:EPAD + L] = x.astype(bf)
    dwr = dw_w.reshape(C, KW).astype(np.float32)
    dww = np.ascontiguousarray(
        dwr.reshape(CT, P, KW).transpose(1, 0, 2).reshape(P, CT * KW),
        dtype=np.float32)
    # diagonal conv blocks for the PE path: dgt[c, (ct*KW+k)*P + c] = w[c,k]
    dgt = np.zeros((P, CT * KW * P), dtype=np.float32)
    idx = np.arange(P)
    for ct in range(CT):
        for k in range(KW):
            dgt[idx, (ct * KW + k) * P + idx] = dwr[ct * P:(ct + 1) * P, k]
    dgt = dgt.astype(bf)
    dwb = np.ascontiguousarray(dw_b.reshape(CT, P).T, dtype=np.float32)
    w1t = np.ascontiguousarray(w1.T.astype(bf))
    b1s = np.ascontiguousarray(b1.reshape(HT, P).T, dtype=np.float32)
    s1s = np.ascontiguousarray(
        w1.astype(np.float32).sum(axis=1).reshape(HT, P).T, dtype=np.float32)
    w2t = np.ascontiguousarray(w2.T.astype(bf))
    b2s = np.ascontiguousarray(b2.reshape(CT, P).T, dtype=np.float32)
    shared = dict(dgt=dgt, dww=dww, dwb=dwb, w1t=w1t, b1s=b1s, s1s=s1s,
                  w2t=w2t, b2s=b2s)
    in_maps = []
    for c in range(N_CORES):
        m = dict(shared)
        m["xb"] = np.ascontiguousarray(xb[c * BPC:(c + 1) * BPC])
        in_maps.append(m)
    return in_maps


def _numpy_fallback(x, dw_w, dw_b, gamma, beta, w1, b1, w2, b2):
    """Pure-host reference path (only used if gamma/beta are non-trivial)."""
    import math
    erf = np.frompyfunc(math.erf, 1, 1)
    x = x.astype(np.float64)
    k = dw_w.reshape(C, KW).astype(np.float64)
    xp = np.pad(x, ((0, 0), (0, 0), (PAD, PAD)))
    p = sum(k[None, :, j:j + 1] * xp[:, :, j:j + L] for j in range(KW))
    p = p + dw_b.astype(np.float64)[None, :, None]
    pt = p.transpose(0, 2, 1)
    mu = pt.mean(axis=(1, 2), keepdims=True)
    var = ((pt - mu) ** 2).mean(axis=(1, 2), keepdims=True)
    n = (pt - mu) / np.sqrt(var + LN_EPS) * gamma.astype(np.float64) \
        + beta.astype(np.float64)
    h = n @ w1.T.astype(np.float64) + b1.astype(np.float64)
    h = 0.5 * h * (1.0 + erf(h / math.sqrt(2.0)).astype(np.float64))
    y = h @ w2.T.astype(np.float64) + b2.astype(np.float64)
    return (y.transpose(0, 2, 1) + x).astype(np.float32)


def kernel(x, dw_w, dw_b, gamma, beta, w1, b1, w2, b2):
    x = np.asarray(x, dtype=np.float32)
    dw_w = np.asarray(dw_w, dtype=np.float32)
    dw_b = np.asarray(dw_b, dtype=np.float32)
    gamma = np.asarray(gamma, dtype=np.float32)
    beta = np.asarray(beta, dtype=np.float32)
    w1 = np.asarray(w1, dtype=np.float32)
    b1 = np.asarray(b1, dtype=np.float32)
    w2 = np.asarray(w2, dtype=np.float32)
    b2 = np.asarray(b2, dtype=np.float32)

    # The device kernel folds LN affine away assuming gamma==1, beta==0
    # (guaranteed by the problem's input spec). Anything else -> host path.
    if not (np.all(gamma == 1.0) and np.all(beta == 0.0)):
        return _numpy_fallback(x, dw_w, dw_b, gamma, beta, w1, b1, w2, b2)

    from concourse.bass_utils import run_bass_kernel_spmd

    nc = _get_program()
    in_maps = _pack_inputs(x, dw_w, dw_b, w1, b1, w2, b2)
    res = run_bass_kernel_spmd(nc, in_maps, list(range(N_CORES)))
    y = np.concatenate([res.results[c]["y"] for c in range(N_CORES)], axis=0)
    return np.ascontiguousarray(y, dtype=np.float32)
